# revision 26
# baseline (speedup 1.0000x reference)
"""Trainium2 Bass kernel for a dense transformer block (B=2, T=2048, C=1024, 16 heads).

Sharding: core = 4*b + g  (b = batch, g = head-group / row-quarter).
  Attention: tensor-parallel over 4 heads per core within each batch group.
  One bf16 ReduceScatter per query chunk turns attn-proj partial sums into
  per-core row shards.  MLP: row-parallel (512 rows per core, full weights).

v3 structure (on top of the v2 baseline):
  - All K=1 bias matmuls removed when the biases are zero (runtime-checked;
    a generic variant with bias matmuls is compiled on demand otherwise).
  - fc split into three row groups: A = rows 0-255 (N=256 moving, fills
    attention c2/c3 as before), B2 = rows 256-383 (N=128, fills chunk-3
    attention after RS(2)), C = rows 384-511 (tail, after RS(3), fused gelu,
    interleaved per-ft with mlp row-block 3).
  - mlp row-blocks 0-2 accumulate during the RS(3) window, interleaved with
    bulk gelu of h rows 0-383 in 8-ft blocks; per-block residual+store.
  - softmax denominators via DVE reciprocal_approx_fast (off the scalar
    engine, which rate-limits attention).
  - host-side re-layout of x / wfc / wmlp / wqkv so streaming DMAs are a
    single contiguous 2D transfer per slab (fewer, bigger DMA issues).
"""

import os
import numpy as np

B, T, C = 2, 2048, 1024
H, D, FF = 16, 64, 4096
N_CORES, G = 8, 4
HPC = H // G               # heads per core
ROWS = T // G              # MLP rows per core
NCT = C // 128             # 8 contraction tiles
CHUNK = 512
NCHUNK = T // CHUNK
NFT = FF // 128            # 32 f-tiles
EPS = 1e-5
QSCALE = float(1.0 / np.sqrt(D))
FB = 256                   # wfc slab width (f columns per slab)
NSLAB = FF // FB           # 8 slabs per fc pass

_cache = {}


def _patch_tile_drain():
    """This walrus build rejects >1 sem-wait on CTRL-class instructions; spread
    the TileContext tail-drain waits across single-wait SP nops."""
    import concourse.tile as tile
    from concourse import mybir
    from concourse.vector_clock import ScopedClock

    if getattr(tile.TileContext, "_drain_patched", False):
        return

    def _drain_and_barrier(self, tick_clock, wait_clock):
        nc = self.nc
        probe = nc.sync.nop()
        wait_clock.add_sem_waits(probe.ins, ScopedClock({None: tick_clock.global_clock}))
        waits = list(probe.ins.sync_info.on_wait) if probe.ins.sync_info else []
        probe.ins.sync_info = mybir.SyncInfo(on_wait=waits[:1], on_update=[])
        for w in waits[1:]:
            nop = nc.sync.nop()
            nop.ins.sync_info = mybir.SyncInfo(on_wait=[w], on_update=[])
        nc.sync.drain()
        nc.all_engine_barrier()
        assert self.sems is not None
        popped = nc._tile_sem_poison_stack.pop()
        assert popped is self._sem_poison
        nc.clear_and_free_semaphores(list(self.sems.allocated().values()))
        nc.all_engine_barrier()

    tile.TileContext._drain_and_barrier = _drain_and_barrier
    tile.TileContext._drain_patched = True


def _split_excess_waits(nc, mybir, maxw=1):
    """walrus in this image rejects instructions carrying more than one sem
    wait; hoist the excess onto same-engine nops placed just before."""
    spare = []

    def make_nop(engine):
        if not spare:
            cur = nc.cur_bb.bb
            n0 = len(cur.instructions)
            for _ in range(64):
                nc.engines[engine].nop()
            insts = list(cur.instructions)
            spare.extend(insts[n0:])
            cur.instructions = insts[:n0]
        n = spare.pop()
        n.engine = engine
        return n

    for f in nc.m.functions:
        for bb in f.blocks:
            insts = list(bb.instructions)
            out = []
            changed = False
            for ins in insts:
                si = ins.sync_info
                if si and si.on_wait and len(si.on_wait) > maxw:
                    waits = list(si.on_wait)
                    for w in waits[maxw:]:
                        nop = make_nop(ins.engine)
                        nop.sync_info = mybir.SyncInfo(on_wait=[w], on_update=[])
                        out.append(nop)
                    ins.sync_info = mybir.SyncInfo(
                        on_wait=waits[:maxw],
                        on_update=list(si.on_update or []))
                    changed = True
                out.append(ins)
            if changed:
                bb.instructions = out


def _build(with_bias):
    import concourse.bass as bass
    import concourse.tile as tile
    from concourse import mybir
    from concourse.masks import make_identity

    _patch_tile_drain()
    dt = mybir.dt
    AF = mybir.ActivationFunctionType
    ALU = mybir.AluOpType

    nc = bass.Bass("TRN2", target_bir_lowering=False, debug=False,
                   num_devices=N_CORES)

    # ---- per-core DRAM parameters (host pre-laid-out; see make_in_maps) ----
    # xT: [128, NCHUNK*NCT*CHUNK] blocks: (chunk, ci) major, contiguous per chunk
    xT_d = nc.dram_tensor("xT", [128, NCHUNK * NCT * CHUNK], dt.bfloat16,
                          kind="ExternalInput")
    # xfull: [128, NCHUNK*4*C] row-major x blocks per (chunk, row-tile)
    xfull_d = nc.dram_tensor("xfull", [128, NCHUNK * 4 * C], dt.bfloat16,
                             kind="ExternalInput")
    # wqkv: [128, NCT*768] ci-major blocks
    wqkv_d = nc.dram_tensor("wqkv", [128, NCT * 3 * 64 * HPC], dt.bfloat16,
                            kind="ExternalInput")
    wproj_d = nc.dram_tensor("wproj", [64 * HPC, C], dt.bfloat16, kind="ExternalInput")
    xrows_d = nc.dram_tensor("xrows", [ROWS, C], dt.float32, kind="ExternalInput")
    # wfc: [128, NSLAB * NCT * FB] blocks: (slab, ci) major
    wfc_d = nc.dram_tensor("wfc", [128, NSLAB * NCT * FB], dt.bfloat16,
                           kind="ExternalInput")
    # wmlp: [128, NFT * C] ft-major blocks
    wmlp_d = nc.dram_tensor("wmlp", [128, NFT * C], dt.bfloat16,
                            kind="ExternalInput")
    cmask_d = nc.dram_tensor("cmask", [128, 2, 128], dt.bfloat16, kind="ExternalInput")
    out_d = nc.dram_tensor("out", [ROWS, C], dt.float32, kind="ExternalOutput")
    if with_bias:
        wqkvb_d = nc.dram_tensor("wqkvb", [1, 3 * 64 * HPC], dt.bfloat16,
                                 kind="ExternalInput")
        bfc_d = nc.dram_tensor("bfc", [1, FF], dt.bfloat16, kind="ExternalInput")
        bmlp_d = nc.dram_tensor("bmlp", [1, C], dt.bfloat16, kind="ExternalInput")

    # internal DRAM for the collective
    cc_in = [nc.dram_tensor(f"cc_in{c}", [CHUNK, C], dt.bfloat16)
             for c in range(NCHUNK)]
    cc_out = [nc.dram_tensor(f"cc_out{c}", [CHUNK // G, C], dt.bfloat16)
              for c in range(NCHUNK)]
    warm_in = nc.dram_tensor("warm_in", [64, 64], dt.bfloat16)
    warm_out = nc.dram_tensor("warm_out", [16, 64], dt.bfloat16)

    with tile.TileContext(nc) as tc:
        with (
            tc.tile_pool(name="persist", bufs=1) as persist,
            tc.tile_pool(name="xsp", bufs=2) as xsp,
            tc.tile_pool(name="statp", bufs=2) as statp,
            tc.tile_pool(name="esp", bufs=2) as esp,
            tc.tile_pool(name="workp", bufs=2) as workp,
            tc.tile_pool(name="wstream", bufs=2) as wstream,
            tc.tile_pool(name="psum", bufs=1, space="PSUM") as psum,
        ):
            # ---------- persistent SBUF ----------
            ident = persist.tile([128, 128], dt.bfloat16)
            ones128 = persist.tile([128, 128], dt.bfloat16)
            ones_row = persist.tile([1, CHUNK], dt.bfloat16)
            eps_col = persist.tile([128, 1], dt.float32)
            cmask_sb = persist.tile([128, 2, 128], dt.bfloat16)
            wqkv_sb = persist.tile([128, NCT, 3 * 64 * HPC], dt.bfloat16)
            wproj_sb = persist.tile([128, 2, C], dt.bfloat16)
            k_sb = persist.tile([128, 2, T], dt.bfloat16)
            v_sb = persist.tile([128, T // 128, (D + 1) * HPC], dt.bfloat16)
            y_sb = persist.tile([128, 2, T], dt.bfloat16)
            x2t_sb = persist.tile([128, G, C], dt.float32)
            x2nT_sb = persist.tile([128, NCT, ROWS], dt.bfloat16)
            h_sb = persist.tile([128, NFT, ROWS], dt.bfloat16)
            if with_bias:
                wqkvb_sb = persist.tile([1, 3 * 64 * HPC], dt.bfloat16)
                bfc_sb = persist.tile([1, FF], dt.bfloat16)
                bmlp_sb = persist.tile([1, C], dt.bfloat16)

            # tiny dummy ReduceScatter fired immediately: absorbs the CC entry
            # barrier + core start-skew before the first real RS
            nc.gpsimd.collective_compute(
                "ReduceScatter", mybir.AluOpType.add,
                replica_groups=[[0, 1, 2, 3], [4, 5, 6, 7]],
                ins=[warm_in.ap().opt()],
                outs=[warm_out.ap().opt()],
            )
            make_identity(nc, ident[:])
            nc.vector.memset(ones128[:], 1.0)
            nc.vector.memset(ones_row[:], 1.0)
            nc.vector.memset(eps_col[:], EPS)
            # stream chunk-0 x first so LN1 stats can start immediately
            xs0 = xsp.tile([128, NCT, CHUNK], dt.bfloat16, tag="xs", name="xs0")
            nc.sync.dma_start(xs0[:].rearrange("p a b -> p (a b)"),
                              xT_d[:, 0:NCT * CHUNK])
            nc.sync.dma_start(cmask_sb[:], cmask_d[:, :, :])
            nc.sync.dma_start(wqkv_sb[:].rearrange("p a b -> p (a b)"),
                              wqkv_d[:, :])
            if with_bias:
                nc.sync.dma_start(wqkvb_sb[:], wqkvb_d[:, :])
                nc.sync.dma_start(bfc_sb[:], bfc_d[:, :])
                nc.sync.dma_start(bmlp_sb[:], bmlp_d[:, :])
            vview = v_sb[:].rearrange("p t (h e) -> p t h e", h=HPC)
            nc.vector.memset(vview[:, :, :, D:D + 1], 1.0)

            WT = ["w0", "w1"]
            wi = [0]

            def wtile(shape, dtype=dt.float32, name=None):
                t = psum.tile(shape, dtype, tag=WT[wi[0] % 2], name=name)
                wi[0] += 1
                return t

            # ---------- MLP prologue: residual + LN2 + transpose ----------
            # split: _dma (issued on the gpsimd queue right after the RS so
            # the sync queue never stalls on an RS-gated DMA), _pre (DVE
            # stats), _chain (scalar rstd + DVE x2n), _tp (PE transposes).
            psh = {}

            def emit_prologue_dma(rt):
                rsl = slice(128 * rt, 128 * (rt + 1))
                xr = workp.tile([128, C], dt.float32, tag="xr", name=f"xr{rt}")
                rsx = workp.tile([128, C], dt.bfloat16, tag="rsx", name=f"rsx{rt}")
                nc.gpsimd.dma_start(xr[:], xrows_d[rsl, :])
                nc.gpsimd.dma_start(rsx[:], cc_out[rt][:, :])
                psh[("dma", rt)] = (xr, rsx)

            def emit_prologue_pre(rt):
                xr, rsx = psh.pop(("dma", rt))
                nc.vector.tensor_add(x2t_sb[:, rt, :], xr[:], rsx[:])
                st = workp.tile([128, 2, 6], dt.float32, tag="st", name=f"st{rt}")
                mv = workp.tile([128, 2], dt.float32, tag="mv", name=f"mv{rt}")
                x2v = x2t_sb[:, rt, :].rearrange("p (s n) -> p s n", s=2)
                for s in range(2):
                    nc.vector.bn_stats(st[:, s, :], x2v[:, s, :])
                nc.vector.bn_aggr(mv[:], st[:])
                psh[rt] = mv

            def emit_prologue_chain(rt):
                mv = psh.pop(rt)
                rstd = workp.tile([128, 1], dt.float32, tag="rstd", name=f"rstd{rt}")
                # 1/std = exp(-0.5*ln(var+eps)) -- stays in the exp table set
                nc.scalar.activation(rstd[:], mv[:, 1:2], AF.Ln,
                                     bias=eps_col[:])
                nc.scalar.activation(rstd[:], rstd[:], AF.Exp, scale=-0.5)
                x2n = workp.tile([128, C], dt.bfloat16, tag="x2n", name=f"x2n{rt}")
                nc.vector.tensor_scalar(x2n[:], x2t_sb[:, rt, :],
                                        mv[:, 0:1], rstd[:],
                                        op0=ALU.subtract, op1=ALU.mult)
                psh[("x2n", rt)] = x2n

            def emit_prologue_tp(rt):
                rsl = slice(128 * rt, 128 * (rt + 1))
                x2n = psh.pop(("x2n", rt))
                for cb in range(NCT):
                    tp = wtile([128, 128], dt.bfloat16, name=f"tp{rt}_{cb}")
                    nc.tensor.transpose(tp[:], x2n[:, 128 * cb:128 * (cb + 1)],
                                        ident[:])
                    nc.vector.tensor_copy(x2nT_sb[:, cb, rsl], tp[:])

            def emit_prologue(rt):
                emit_prologue_pre(rt)
                emit_prologue_chain(rt)
                emit_prologue_tp(rt)

            # ---------- MLP fc tiles over an arbitrary row group ----------
            # rows [r0, r0+nw); moving width nw in {128, 256}
            fc_hold = {}

            def fc_load_w(key, sb):
                wf = wstream.tile([128, NCT, FB], dt.bfloat16, tag="wf", bufs=4,
                                  name=f"wf{key}_{sb}")
                nc.sync.dma_start(wf[:].rearrange("p a b -> p (a b)"),
                                  wfc_d[:, NCT * FB * sb:NCT * FB * (sb + 1)])
                fc_hold[(key, sb)] = wf

            FPS = FB // 128    # ft tiles per slab
            b2_done = [0]      # fc-B ft-completion watermark (for tail gelu)

            def fc_tile(key, r0, nw, ft, fused_gelu):
                if key == "B":
                    b2_done[0] = ft + 1
                wf = fc_hold[(key, ft // FPS)]
                rsl = slice(r0, r0 + nw)
                csl = slice(128 * (ft % FPS), 128 * (ft % FPS + 1))
                ps = wtile([128, nw], name=f"fc{key}_{ft}")
                for ci in range(NCT):
                    nc.tensor.matmul(ps[:], wf[:, ci, csl],
                                     x2nT_sb[:, ci, rsl],
                                     start=(ci == 0),
                                     stop=(ci == NCT - 1 and not with_bias))
                if with_bias:
                    nc.tensor.matmul(ps[:],
                                     bfc_sb[0:1, 128 * ft:128 * (ft + 1)],
                                     ones_row[0:1, 0:nw],
                                     start=False, stop=True)
                if fused_gelu:
                    nc.scalar.activation(h_sb[:, ft, rsl], ps[:], AF.Gelu)
                else:
                    nc.vector.tensor_copy(h_sb[:, ft, rsl], ps[:])

            def emit_fc_group(key, r0, nw, defer=None, fused_gelu=False):
                fc_load_w(key, 0)    # issue the first weight-slab DMA right away
                for ft in range(NFT):
                    def step(ft=ft):
                        if ft % FPS == 0:
                            for nxt in (ft // FPS + 1, ft // FPS + 2):
                                if nxt < NSLAB and (key, nxt) not in fc_hold:
                                    fc_load_w(key, nxt)   # prefetch ahead
                        fc_tile(key, r0, nw, ft, fused_gelu)
                    if defer is None:
                        step()
                    else:
                        defer.append(step)

            # ---------- MLP proj weight stream: one slab per 2 ft ----------
            wm_hold = {}

            def load_wm(key, ftp):
                wm = wstream.tile([128, 2, C], dt.bfloat16, tag="wm", bufs=6,
                                  name=f"wm{key}_{ftp}")
                nc.sync.dma_start(wm[:].rearrange("p a b -> p (a b)"),
                                  wmlp_d[:, 2 * C * ftp:2 * C * (ftp + 1)])
                wm_hold[(key, ftp)] = wm

            def wm_get(key, ft):
                ftp = ft // 2
                if (key, ftp) not in wm_hold:
                    load_wm(key, ftp)
                if ftp + 2 < NFT // 2 and (key, ftp + 2) not in wm_hold:
                    load_wm(key, ftp + 2)     # prefetch 2 slabs ahead
                wm = wm_hold[(key, ftp)]
                if ft % 2 == 1:
                    del wm_hold[(key, ftp)]
                return wm[:, ft % 2, :]

            # ---------- per-row-block MLP proj + residual + store ----------
            def mlp_mm(mps, rt, ft, wmrow):
                for cc in range(2):
                    csl = slice(512 * cc, 512 * (cc + 1))
                    nc.tensor.matmul(mps[cc], h_sb[:, ft, 128 * rt:128 * (rt + 1)],
                                     wmrow[:, csl],
                                     start=(ft == 0),
                                     stop=(ft == NFT - 1 and not with_bias))

            def mlp_fin(mps, rt):
                for cc in range(2):
                    csl = slice(512 * cc, 512 * (cc + 1))
                    if with_bias:
                        nc.tensor.matmul(mps[cc], ones128[0:1, :],
                                         bmlp_sb[0:1, csl],
                                         start=False, stop=True)
                    fin = workp.tile([128, 512], dt.float32, tag="fin",
                                     name=f"fin{rt}_{cc}")
                    nc.vector.tensor_add(fin[:], mps[cc],
                                         x2t_sb[:, rt, csl])
                    nc.gpsimd.dma_start(out_d[slice(128 * rt, 128 * (rt + 1)), csl],
                                        fin[:])

            # mlp accumulators live on the attention-dead sA/sB/y0/y1 PSUM
            # tags so the w0/w1 wtile rotation stays free for fc tiles that
            # interleave with the open mlp accumulation groups
            def alloc_mp2(tag, name):
                t = psum.tile([128, 2, 512], dt.float32, tag=tag, name=name)
                return [t[:, 0, :], t[:, 1, :]]

            # Two fill queues popped into attention units: qkv_q (stats/xn/qkv
            # steps of the NEXT chunk -- must drain before that chunk's
            # attention) and fill_q (fc tiles -- fully opportunistic).
            qkv_q = []
            fill_q = []
            qtiles = {}

            def pop_fill():
                if qkv_q:
                    qkv_q.pop(0)()
                elif fill_q:
                    fill_q.pop(0)()

            def stage_chunk(c, xs):
                """Append LN1-stats + qkv emission steps for chunk c.

                LN1 stats come from DVE bn_stats on row-major x tiles (xfull);
                the per-row mean/rstd columns are transposed once on the PE and
                broadcast to [128, T] via K=1 matmuls.  In the no-bias fast
                path the rstd scale rides on the PSUM->SBUF copies, so the
                in-place xn only subtracts the mean."""
                isl = slice(CHUNK * c, CHUNK * (c + 1))
                sh = {}
                xfs = []
                for tt4 in range(4):
                    xf = statp.tile([128, C], dt.bfloat16, tag="xf", bufs=4,
                                    name=f"xf{c}_{tt4}")
                    nc.sync.dma_start(
                        xf[:], xfull_d[:, 4 * C * c + C * tt4:
                                       4 * C * c + C * (tt4 + 1)])
                    xfs.append(xf)
                qt = statp.tile([128, 2, CHUNK], dt.bfloat16, tag="qt",
                                bufs=2, name=f"qt{c}")
                qtiles[c] = qt
                pack_mu = statp.tile([128, 128], dt.bfloat16, tag="pkm",
                                     bufs=2, name=f"pkm{c}")
                pack_rs = statp.tile([128, 128], dt.bfloat16, tag="pkr",
                                     bufs=2, name=f"pkr{c}")
                rcol = statp.tile([128, 4], dt.float32, tag="rcol", bufs=2,
                                  name=f"rcol{c}")

                def bn_step(tt4):
                    xf = xfs[tt4]
                    st = statp.tile([128, 2, 6], dt.float32, tag="bst", bufs=2,
                                    name=f"bst{c}_{tt4}")
                    mv = statp.tile([128, 2], dt.float32, tag="bmv", bufs=2,
                                    name=f"bmv{c}_{tt4}")
                    xfv = xf[:].rearrange("p (s n) -> p s n", s=2)
                    for sb2 in range(2):
                        nc.vector.bn_stats(st[:, sb2, :], xfv[:, sb2, :])
                    nc.vector.bn_aggr(mv[:], st[:])
                    # rstd = exp(-0.5*ln(var+eps)) -- stays in the exp table
                    nc.scalar.activation(rcol[:, tt4:tt4 + 1], mv[:, 1:2],
                                         AF.Ln, bias=eps_col[:])
                    nc.scalar.activation(rcol[:, tt4:tt4 + 1],
                                         rcol[:, tt4:tt4 + 1], AF.Exp,
                                         scale=-0.5)
                    nc.vector.tensor_copy(
                        pack_mu[:, 32 * tt4:32 * tt4 + 1], mv[:, 0:1])
                    nc.vector.tensor_copy(
                        pack_rs[:, 32 * tt4:32 * tt4 + 1],
                        rcol[:, tt4:tt4 + 1])

                def tb_step():
                    tpm = wtile([128, 128], dt.bfloat16, name=f"stm{c}")
                    nc.tensor.transpose(tpm[:], pack_mu[:], ident[:])
                    tpr = wtile([128, 128], dt.bfloat16, name=f"str{c}")
                    nc.tensor.transpose(tpr[:], pack_rs[:], ident[:])
                    srow = statp.tile([1, 8, 128], dt.bfloat16, tag="srow",
                                      bufs=2, name=f"srow{c}")
                    for tt4 in range(4):
                        nc.vector.tensor_copy(srow[0:1, 2 * tt4, :],
                                              tpm[32 * tt4:32 * tt4 + 1, :])
                        nc.vector.tensor_copy(srow[0:1, 2 * tt4 + 1, :],
                                              tpr[32 * tt4:32 * tt4 + 1, :])
                    mu_ps = wtile([128, CHUNK], name=f"mups{c}")
                    rs_ps = wtile([128, CHUNK], name=f"rsps{c}")
                    for tt4 in range(4):
                        csl = slice(128 * tt4, 128 * (tt4 + 1))
                        nc.tensor.matmul(mu_ps[:, csl], ones128[0:1, :],
                                         srow[0:1, 2 * tt4, :],
                                         start=True, stop=True,
                                         skip_group_check=True)
                        nc.tensor.matmul(rs_ps[:, csl], ones128[0:1, :],
                                         srow[0:1, 2 * tt4 + 1, :],
                                         start=True, stop=True,
                                         skip_group_check=True)
                    mu_t = statp.tile([128, CHUNK], dt.bfloat16, tag="mu",
                                      name=f"mu{c}")
                    rsig = statp.tile([128, CHUNK], dt.bfloat16, tag="rsig",
                                      name=f"rsig{c}")
                    nc.vector.tensor_copy(mu_t[:], mu_ps[:])
                    nc.vector.tensor_copy(rsig[:], rs_ps[:])
                    sh["mu"], sh["rsig"] = mu_t, rsig

                def xn_step(g):             # 2 steps x 4 ci, in-place
                    for ci in range(4 * g, 4 * g + 4):
                        nc.vector.tensor_sub(xs[:, ci, :], xs[:, ci, :],
                                             sh["mu"][:])
                        if with_bias:
                            nc.vector.tensor_mul(xs[:, ci, :], xs[:, ci, :],
                                                 sh["rsig"][:])

                def qk_step(blk, s):
                    dsl = (slice(None), s, slice(0, CHUNK)) if blk == 0 \
                        else (slice(None), s, isl)
                    dst = qt if blk == 0 else k_sb
                    cols = slice(256 * blk + 128 * s, 256 * blk + 128 * (s + 1))
                    ps = wtile([128, CHUNK], name=f"qk{c}_{blk}_{s}")
                    for ci in range(NCT):
                        nc.tensor.matmul(ps[:], wqkv_sb[:, ci, cols],
                                         xs[:, ci, :],
                                         start=(ci == 0),
                                         stop=(ci == NCT - 1 and not with_bias))
                    if with_bias:
                        nc.tensor.matmul(ps[:], wqkvb_sb[0:1, cols],
                                         ones_row[0:1, :],
                                         start=False, stop=True)
                        nc.vector.tensor_copy(dst[dsl], ps[:])
                    else:
                        nc.vector.tensor_mul(dst[dsl], ps[:],
                                             sh["rsig"][:])

                def v_step(tt4):
                    tt = 4 * c + tt4
                    tsl = slice(128 * tt4, 128 * (tt4 + 1))
                    ps = wtile([128, 64 * HPC], name=f"v{c}_{tt4}")
                    for ci in range(NCT):
                        nc.tensor.matmul(ps[:], xs[:, ci, tsl],
                                         wqkv_sb[:, ci, 512:768],
                                         start=(ci == 0),
                                         stop=(ci == NCT - 1 and not with_bias))
                    if with_bias:
                        nc.tensor.matmul(ps[:], ones_row[0:1, 0:128],
                                         wqkvb_sb[0:1, 512:768],
                                         start=False, stop=True)
                        nc.vector.tensor_copy(
                            vview[:, tt, :, 0:D],
                            ps[:].rearrange("p (h e) -> p h e", e=D))
                    else:
                        nc.vector.tensor_scalar_mul(
                            vview[:, tt, :, 0:D],
                            ps[:].rearrange("p (h e) -> p h e", e=D),
                            rcol[:, tt4:tt4 + 1])

                for tt4 in range(4):
                    qkv_q.append(lambda tt4=tt4: bn_step(tt4))
                qkv_q.append(tb_step)
                for g in range(2):
                    qkv_q.append(lambda g=g: xn_step(g))
                for blk in (0, 1):
                    for s in range(2):
                        qkv_q.append(lambda blk=blk, s=s: qk_step(blk, s))
                for tt4 in range(CHUNK // 128):
                    qkv_q.append(lambda tt4=tt4: v_step(tt4))

            # a few junk matmuls during the initial x DMA: ~3.5us of PE busy
            # flips the HAM clock gate to its fast state before real work
            warm_ps = psum.tile([128, CHUNK], dt.float32, tag="w0",
                                name="warm_ps")
            for _ in range(12):
                nc.tensor.matmul(warm_ps[:], ones128[0:1, :], ones_row[0:1, :],
                                 start=True, stop=True)

            # =======================  main chunk loop  =======================
            for c in range(NCHUNK):
                isl = slice(CHUNK * c, CHUNK * (c + 1))

                # chunk c's stats/qkv must be fully emitted before its attention
                if c == 0:
                    # interleave chunk-0 and chunk-1 qkv pipelines so chunk-1
                    # stats matmuls fill chunk-0's stats->chain->xn latency.
                    # Order is deadlock-safe wrt the w0/w1 PSUM tag rotation:
                    # B's chain (B4) must be emitted before A's qk/v wtiles.
                    stage_chunk(0, xs0)
                    qA = list(qkv_q); qkv_q.clear()
                    xs1 = xsp.tile([128, NCT, CHUNK], dt.bfloat16, tag="xs",
                                   name="xs1")
                    nc.sync.dma_start(xs1[:].rearrange("p a b -> p (a b)"),
                                      xT_d[:, NCT * CHUNK:2 * NCT * CHUNK])
                    nc.sync.dma_start(wproj_sb[:, 0, :], wproj_d[0:128, :])
                    nc.sync.dma_start(wproj_sb[:, 1, :], wproj_d[128:256, :])
                    stage_chunk(1, xs1)
                    qB = list(qkv_q); qkv_q.clear()
                    for step in (qA[0:5] + qB[0:2] + qA[5:7] + qB[2:5]
                                 + qA[7:15]):
                        step()
                    qkv_q.extend(qB[5:])
                while qkv_q:
                    qkv_q.pop(0)()

                # stage the NEXT chunk: stream its x^T, queue its steps
                if c + 1 < NCHUNK and c >= 1:
                    xs_n = xsp.tile([128, NCT, CHUNK], dt.bfloat16, tag="xs",
                                    name=f"xs{c + 1}")
                    nc.sync.dma_start(
                        xs_n[:].rearrange("p a b -> p (a b)"),
                        xT_d[:, NCT * CHUNK * (c + 1):NCT * CHUNK * (c + 2)])
                    stage_chunk(c + 1, xs_n)

                # ---- attention for this chunk ----
                njt = 4 * c + 4
                for hp in range(2):
                    # at attn(2) hp1, RS(0) and RS(1) have finished: emit
                    # prologues 0+1 and start filling units with fc-A tiles
                    if c == 2 and hp == 1:
                        emit_prologue(0)
                        emit_prologue(1)
                        emit_fc_group("A", 0, 256, defer=fill_q)
                    ys = [psum.tile([D + 1, CHUNK], dt.float32, tag=f"y{u}",
                                    name=f"ys{c}_{hp}_{u}") for u in range(2)]
                    for jt in range(njt):
                        # late hp0: RS(2) is done -> prologue(2) + fc-B2
                        # tiles for rows 256-383 keep the fill queue stocked
                        if c == 3 and hp == 0 and jt == 12:
                            emit_prologue(2)
                            emit_fc_group("B", 256, 128, defer=fill_q)
                        # prebuffer mlp-proj weight slabs for the tail, two
                        # at a time so the slab feed for fills never starves
                        if c == 3 and hp == 1 and jt in (0, 6, 12):
                            load_wm("m", jt // 3)
                            load_wm("m", jt // 3 + 1)
                        jsl = slice(128 * jt, 128 * (jt + 1))
                        off = 128 * (jt - 4 * c) if jt >= 4 * c else 0
                        sp = psum.tile([128, 2, CHUNK], dt.float32,
                                       tag=("sA" if jt % 2 == 0 else "sB"),
                                       name=f"sp{c}_{hp}_{jt}")
                        for u in range(2):
                            r = slice(64 * u, 64 * (u + 1))
                            nc.tensor.matmul(
                                sp[:, u, off:CHUNK],
                                k_sb[r, hp, jsl],
                                qtiles[c][r, hp, off:CHUNK],
                                start=True, stop=True,
                                tile_position=(64 * u, 0))
                        es = esp.tile([128, 2, CHUNK], dt.bfloat16, tag="es",
                                      name=f"es{c}_{hp}_{jt}")
                        nc.scalar.activation(es[:, :, off:CHUNK],
                                             sp[:, :, off:CHUNK], AF.Exp)
                        if jt >= 4 * c:
                            nc.vector.tensor_mul(es[:, :, off:off + 128],
                                                 es[:, :, off:off + 128],
                                                 cmask_sb[:])
                        for u in range(2):
                            h_ = 2 * hp + u
                            nc.tensor.matmul(
                                ys[u][:, off:CHUNK],
                                v_sb[:, jt, (D + 1) * h_:(D + 1) * (h_ + 1)],
                                es[:, u, off:CHUNK],
                                start=(jt == 0), stop=(jt == njt - 1),
                                skip_group_check=True)
                        pop_fill()   # interleave a staged qkv/fc step
                    for u in range(2):
                        ysb = workp.tile([D + 1, CHUNK], dt.bfloat16, tag="ysb",
                                         name=f"ysb{c}_{hp}_{u}")
                        nc.vector.tensor_copy(ysb[:], ys[u][:])
                        # 1/denom = exp(-ln(d)) on the [1,512] row (exp-set
                        # resident), then matmul-broadcast across partitions
                        rln = workp.tile([D + 1, CHUNK], dt.float32, tag="rln", bufs=1,
                                         name=f"rln{c}_{hp}_{u}")
                        nc.scalar.activation(rln[D:D + 1, :], ysb[D:D + 1, :],
                                             AF.Ln)
                        rinv = workp.tile([D + 1, CHUNK], dt.bfloat16, tag="rinv", bufs=1,
                                          name=f"rinv{c}_{hp}_{u}")
                        nc.scalar.activation(rinv[D:D + 1, :], rln[D:D + 1, :],
                                             AF.Exp, scale=-1.0)
                        dbc = wtile([128, CHUNK], name=f"dbc{c}_{hp}_{u}")
                        nc.tensor.matmul(dbc[:], ones128[D:D + 1, :],
                                         rinv[D:D + 1, :], start=True, stop=True)
                        nc.vector.tensor_mul(y_sb[64 * u:64 * (u + 1), hp, isl],
                                             ysb[0:D, :], dbc[0:D, :])

                # ---- attention proj for this chunk ----
                for tt4 in range(CHUNK // 128):
                    t0 = CHUNK * c + 128 * tt4
                    for cc in range(2):
                        csl = slice(512 * cc, 512 * (cc + 1))
                        pp = wtile([128, 512], name=f"pp{c}_{tt4}_{cc}")
                        for hp in range(2):
                            nc.tensor.matmul(pp[:], y_sb[:, hp, t0:t0 + 128],
                                             wproj_sb[:, hp, csl],
                                             start=(hp == 0), stop=(hp == 1))
                        ob = workp.tile([128, 512], dt.bfloat16, tag="ob",
                                        name=f"ob{c}_{tt4}_{cc}")
                        nc.vector.tensor_copy(ob[:], pp[:])
                        nc.gpsimd.dma_start(
                            cc_in[c][128 * tt4:128 * (tt4 + 1), csl], ob[:])

                # ---- ReduceScatter for this chunk's rows ----
                nc.gpsimd.collective_compute(
                    "ReduceScatter", mybir.AluOpType.add,
                    replica_groups=[[0, 1, 2, 3], [4, 5, 6, 7]],
                    ins=[cc_in[c].ap().opt()],
                    outs=[cc_out[c].ap().opt()],
                )
                # pre-issue this chunk's prologue DMAs right behind the RS:
                # they carry the RS sem-wait, and nothing urgent sits behind
                # them on the gpsimd queue
                emit_prologue_dma(c)

            # ============================  tail  ============================
            # mlp row-blocks 0-2 interleaved with the remaining fc-B2 fills
            # (spreads their slab-DMA demand) and with bulk gelu of rows
            # 0-383 in 8-ft blocks; all of this runs during RS(3).
            mps012 = [alloc_mp2("sA", "mp0"), alloc_mp2("sB", "mp1"),
                      [psum.tile([128, 512], dt.float32, tag="y0", name="mp2_0"),
                       psum.tile([128, 512], dt.float32, tag="y1", name="mp2_1")]]

            def drain_b2(ft_needed):
                while fill_q and b2_done[0] < ft_needed:
                    fill_q.pop(0)()

            drain_b2(8)
            nc.scalar.activation(h_sb[:, 0:8, 0:384], h_sb[:, 0:8, 0:384],
                                 AF.Gelu)
            for ft in range(NFT):
                if fill_q:
                    fill_q.pop(0)()
                if ft % 8 == 7 and ft < NFT - 1:
                    fb = ft // 8 + 1
                    drain_b2(8 * (fb + 1))
                    nc.scalar.activation(h_sb[:, 8 * fb:8 * (fb + 1), 0:384],
                                         h_sb[:, 8 * fb:8 * (fb + 1), 0:384],
                                         AF.Gelu)
                if ft == 26:
                    fc_load_w("C", 0)
                    fc_load_w("C", 1)
                if ft == 29:
                    load_wm("m3", 0)
                    load_wm("m3", 1)
                wmrow = wm_get("m", ft)
                for rt in range(3):
                    mlp_mm(mps012[rt], rt, ft, wmrow)
            while fill_q:
                fill_q.pop(0)()
            for rt in range(3):
                mlp_fin(mps012[rt], rt)
            # prologue(3) DVE stats: emitted only now so the RS(3)-gated data
            # dependency never stalls the B2 h-copies in the DVE FIFO
            emit_prologue_pre(3)
            # rows 384-511: LN2 chain (exp-set reload), transposes, then fc-C
            # with fused gelu interleaved per-ft with mlp row-block 3.
            emit_prologue_chain(3)
            emit_prologue_tp(3)
            mps3 = alloc_mp2("sA", "mp3")
            for ft in range(NFT):
                if ft % FPS == 0:
                    for nxt in (ft // FPS + 1, ft // FPS + 2):
                        if nxt < NSLAB and ("C", nxt) not in fc_hold:
                            fc_load_w("C", nxt)
                fc_tile("C", 384, 128, ft, fused_gelu=True)
                wmrow = wm_get("m3", ft)
                mlp_mm(mps3, 3, ft, wmrow)
            mlp_fin(mps3, 3)

    _split_excess_waits(nc, mybir)
    return nc


def _get_nc(with_bias):
    key = ("nc", with_bias)
    if key not in _cache:
        _cache[key] = _build(with_bias)
    return _cache[key]


def make_in_maps(inputs, with_bias):
    import ml_dtypes
    bf16 = ml_dtypes.bfloat16
    x = np.asarray(inputs["x"], np.float32)
    w_qkv = np.asarray(inputs["w_qkv"], np.float32)
    w_attn_proj = np.asarray(inputs["w_attn_proj"], np.float32)
    ln1_w = np.asarray(inputs["ln1_w"], np.float32)
    ln1_b = np.asarray(inputs["ln1_b"], np.float32)
    ln2_w = np.asarray(inputs["ln2_w"], np.float32)
    ln2_b = np.asarray(inputs["ln2_b"], np.float32)
    w_fc = np.asarray(inputs["w_fc"], np.float32)
    b_fc = np.asarray(inputs["b_fc"], np.float32)
    w_mlp_proj = np.asarray(inputs["w_mlp_proj"], np.float32)
    b_mlp_proj = np.asarray(inputs["b_mlp_proj"], np.float32)

    wfc_in = (ln2_w[:, None] * w_fc).astype(bf16)           # [C, FF]
    # wfc re-layout: [128, (slab, ci, fb)]  slab = f // FB
    wfc_r = wfc_in.reshape(NCT, 128, NSLAB, FB)             # [ci, p, slab, fb]
    wfc_r = np.ascontiguousarray(wfc_r.transpose(1, 2, 0, 3)).reshape(128, -1)
    # wmlp re-layout: [128, (ft, c)]
    wmlp_r = w_mlp_proj.astype(bf16).reshape(NFT, 128, C)
    wmlp_r = np.ascontiguousarray(wmlp_r.transpose(1, 0, 2)).reshape(128, -1)

    jj = np.arange(128)[:, None]
    ii = np.arange(128)[None, :]
    cm1 = (ii >= jj).astype(np.float32)
    cmask = np.stack([cm1, cm1], axis=1).astype(bf16)   # [128, 2, 128]

    in_maps = []
    for core in range(N_CORES):
        b, g = divmod(core, G)
        hsl = slice(256 * g, 256 * (g + 1))
        raw768 = np.concatenate([w_qkv[:, :C][:, hsl] * QSCALE,
                                 w_qkv[:, C:2 * C][:, hsl],
                                 w_qkv[:, 2 * C:][:, hsl]], axis=1)
        rowidx = np.concatenate([np.arange(CHUNK * c + 128 * g,
                                           CHUNK * c + 128 * (g + 1))
                                 for c in range(NCHUNK)])
        # xT re-layout: [128, (chunk, ci, t)]
        xT = x[b].T.astype(bf16)                            # [C, T]
        xT_r = xT.reshape(NCT, 128, NCHUNK, CHUNK)          # [ci, p, c, t]
        xT_r = np.ascontiguousarray(xT_r.transpose(1, 2, 0, 3)).reshape(128, -1)
        # xfull re-layout: [128, (chunk, row-tile, C)] row-major x
        xfull_r = x[b].astype(bf16).reshape(NCHUNK * 4, 128, C)
        xfull_r = np.ascontiguousarray(
            xfull_r.transpose(1, 0, 2)).reshape(128, -1)
        # wqkv re-layout: [128, (ci, col)]
        wq = (ln1_w[:, None] * raw768).astype(bf16)         # [C, 768]
        wq_r = np.ascontiguousarray(
            wq.reshape(NCT, 128, 3 * 64 * HPC).transpose(1, 0, 2)).reshape(128, -1)
        m = {
            "cmask": cmask,
            "xT": xT_r,
            "xfull": xfull_r,
            "wqkv": wq_r,
            "wproj": w_attn_proj[hsl, :].astype(bf16),
            "xrows": np.ascontiguousarray(x[b, rowidx, :]),
            "wfc": wfc_r,
            "wmlp": wmlp_r,
        }
        if with_bias:
            m["wqkvb"] = (ln1_b @ raw768)[None, :].astype(bf16)
            m["bfc"] = (b_fc + ln2_b @ w_fc)[None, :].astype(bf16)
            m["bmlp"] = b_mlp_proj[None, :].astype(bf16)
        in_maps.append(m)
    return in_maps


def assemble_out(results):
    out = np.empty((B, T, C), np.float32)
    for core in range(N_CORES):
        b, g = divmod(core, G)
        for c in range(NCHUNK):
            out[b, CHUNK * c + 128 * g:CHUNK * c + 128 * (g + 1), :] = \
                results[core]["out"][128 * c:128 * (c + 1), :]
    return out


def _setup_trace_shims():
    """Register the NTFF profile hook (missing antenv.axon_hooks on this image)
    so run_bass_kernel_spmd(trace=True) can report exec_time_ns."""
    import sys, types
    if "antenv.axon_hooks" not in sys.modules:
        mod = types.ModuleType("antenv.axon_hooks")
        hook = {}
        mod.set_axon_ntff_profile_hook = lambda h: hook.__setitem__("h", h)
        mod.get_axon_ntff_profile_hook = lambda: hook.get("h")
        sys.modules["antenv.axon_hooks"] = mod
        try:
            from trn_agent_boot.trn_boot import _ntff_profile_via_ctypes
            mod.set_axon_ntff_profile_hook(
                _ntff_profile_via_ctypes("/opt/axon/libaxon_pjrt.so"))
        except Exception:
            pass
    import concourse.bass_utils as bu
    bu.upload_artifacts = lambda tmpdir: tmpdir


def kernel(**inputs):
    trace = bool(os.environ.get("KERNEL_TRACE"))
    if trace:
        _setup_trace_shims()
    from concourse.bass_utils import run_bass_kernel_spmd

    with_bias = not all(
        float(np.abs(np.asarray(inputs[k], np.float32)).max()) == 0.0
        for k in ("ln1_b", "ln2_b", "b_fc", "b_mlp_proj"))
    in_maps = make_in_maps(inputs, with_bias)
    nc = _get_nc(with_bias)
    res = run_bass_kernel_spmd(nc, in_maps, core_ids=list(range(N_CORES)),
                               trace=trace)
    if trace:
        _cache["exec_time_ns"] = res.exec_time_ns
    return assemble_out(res.results)


if __name__ == "__main__":
    nc = _get_nc(False)
    print("built OK; instructions:", len(nc.inst_map))


# revision 27
# speedup vs baseline: 1.0438x; 1.0438x over previous
"""Trainium2 Bass kernel for a dense transformer block (B=2, T=2048, C=1024, 16 heads).

Sharding: core = 4*b + g  (b = batch, g = head-group / row-quarter).
  Attention: tensor-parallel over 4 heads per core within each batch group.
  One bf16 ReduceScatter per query chunk turns attn-proj partial sums into
  per-core row shards.  MLP: row-parallel (512 rows per core, full weights).

v3 structure (on top of the v2 baseline):
  - All K=1 bias matmuls removed when the biases are zero (runtime-checked;
    a generic variant with bias matmuls is compiled on demand otherwise).
  - fc split into three row groups: A = rows 0-255 (N=256 moving, fills
    attention c2/c3 as before), B2 = rows 256-383 (N=128, fills chunk-3
    attention after RS(2)), C = rows 384-511 (tail, after RS(3), fused gelu,
    interleaved per-ft with mlp row-block 3).
  - mlp row-blocks 0-2 accumulate during the RS(3) window, interleaved with
    bulk gelu of h rows 0-383 in 8-ft blocks; per-block residual+store.
  - softmax denominators via DVE reciprocal_approx_fast (off the scalar
    engine, which rate-limits attention).
  - host-side re-layout of x / wfc / wmlp / wqkv so streaming DMAs are a
    single contiguous 2D transfer per slab (fewer, bigger DMA issues).
"""

import os
import numpy as np

B, T, C = 2, 2048, 1024
H, D, FF = 16, 64, 4096
N_CORES, G = 8, 4
HPC = H // G               # heads per core
ROWS = T // G              # MLP rows per core
NCT = C // 128             # 8 contraction tiles
CHUNK = 512
NCHUNK = T // CHUNK
NFT = FF // 128            # 32 f-tiles
EPS = 1e-5
QSCALE = float(1.0 / np.sqrt(D))
FB = 256                   # wfc slab width (f columns per slab)
NSLAB = FF // FB           # 8 slabs per fc pass

_cache = {}


def _patch_tile_drain():
    """This walrus build rejects >1 sem-wait on CTRL-class instructions; spread
    the TileContext tail-drain waits across single-wait SP nops."""
    import concourse.tile as tile
    from concourse import mybir
    from concourse.vector_clock import ScopedClock

    if getattr(tile.TileContext, "_drain_patched", False):
        return

    def _drain_and_barrier(self, tick_clock, wait_clock):
        nc = self.nc
        probe = nc.sync.nop()
        wait_clock.add_sem_waits(probe.ins, ScopedClock({None: tick_clock.global_clock}))
        waits = list(probe.ins.sync_info.on_wait) if probe.ins.sync_info else []
        probe.ins.sync_info = mybir.SyncInfo(on_wait=waits[:1], on_update=[])
        for w in waits[1:]:
            nop = nc.sync.nop()
            nop.ins.sync_info = mybir.SyncInfo(on_wait=[w], on_update=[])
        nc.sync.drain()
        nc.all_engine_barrier()
        assert self.sems is not None
        popped = nc._tile_sem_poison_stack.pop()
        assert popped is self._sem_poison
        nc.clear_and_free_semaphores(list(self.sems.allocated().values()))
        nc.all_engine_barrier()

    tile.TileContext._drain_and_barrier = _drain_and_barrier
    tile.TileContext._drain_patched = True


def _split_excess_waits(nc, mybir, maxw=1):
    """walrus in this image rejects instructions carrying more than one sem
    wait; hoist the excess onto same-engine nops placed just before."""
    spare = []

    def make_nop(engine):
        if not spare:
            cur = nc.cur_bb.bb
            n0 = len(cur.instructions)
            for _ in range(64):
                nc.engines[engine].nop()
            insts = list(cur.instructions)
            spare.extend(insts[n0:])
            cur.instructions = insts[:n0]
        n = spare.pop()
        n.engine = engine
        return n

    for f in nc.m.functions:
        for bb in f.blocks:
            insts = list(bb.instructions)
            out = []
            changed = False
            for ins in insts:
                si = ins.sync_info
                if si and si.on_wait and len(si.on_wait) > maxw:
                    waits = list(si.on_wait)
                    for w in waits[maxw:]:
                        nop = make_nop(ins.engine)
                        nop.sync_info = mybir.SyncInfo(on_wait=[w], on_update=[])
                        out.append(nop)
                    ins.sync_info = mybir.SyncInfo(
                        on_wait=waits[:maxw],
                        on_update=list(si.on_update or []))
                    changed = True
                out.append(ins)
            if changed:
                bb.instructions = out


def _build(with_bias):
    import concourse.bass as bass
    import concourse.tile as tile
    from concourse import mybir
    from concourse.masks import make_identity

    _patch_tile_drain()
    dt = mybir.dt
    AF = mybir.ActivationFunctionType
    ALU = mybir.AluOpType

    nc = bass.Bass("TRN2", target_bir_lowering=False, debug=False,
                   num_devices=N_CORES)

    # ---- per-core DRAM parameters (host pre-laid-out; see make_in_maps) ----
    # xT: [128, NCHUNK*NCT*CHUNK] blocks: (chunk, ci) major, contiguous per chunk
    xT_d = nc.dram_tensor("xT", [128, NCHUNK * NCT * CHUNK], dt.bfloat16,
                          kind="ExternalInput")
    # xfull: [128, NCHUNK*4*C] row-major x blocks per (chunk, row-tile)
    xfull_d = nc.dram_tensor("xfull", [128, NCHUNK * 4 * C], dt.bfloat16,
                             kind="ExternalInput")
    # wqkv: [128, NCT*768] ci-major blocks
    wqkv_d = nc.dram_tensor("wqkv", [128, NCT * 3 * 64 * HPC], dt.bfloat16,
                            kind="ExternalInput")
    wproj_d = nc.dram_tensor("wproj", [64 * HPC, C], dt.bfloat16, kind="ExternalInput")
    xrows_d = nc.dram_tensor("xrows", [ROWS, C], dt.float32, kind="ExternalInput")
    # wfc: [128, NSLAB * NCT * FB] blocks: (slab, ci) major
    wfc_d = nc.dram_tensor("wfc", [128, NSLAB * NCT * FB], dt.bfloat16,
                           kind="ExternalInput")
    # wmlp: [128, NFT * C] ft-major blocks
    wmlp_d = nc.dram_tensor("wmlp", [128, NFT * C], dt.bfloat16,
                            kind="ExternalInput")
    cmask_d = nc.dram_tensor("cmask", [128, 2, 128], dt.bfloat16, kind="ExternalInput")
    out_d = nc.dram_tensor("out", [ROWS, C], dt.float32, kind="ExternalOutput")
    if with_bias:
        wqkvb_d = nc.dram_tensor("wqkvb", [1, 3 * 64 * HPC], dt.bfloat16,
                                 kind="ExternalInput")
        bfc_d = nc.dram_tensor("bfc", [1, FF], dt.bfloat16, kind="ExternalInput")
        bmlp_d = nc.dram_tensor("bmlp", [1, C], dt.bfloat16, kind="ExternalInput")

    # internal DRAM for the collective
    cc_in = [nc.dram_tensor(f"cc_in{c}", [CHUNK, C], dt.bfloat16)
             for c in range(NCHUNK)]
    cc_out = [nc.dram_tensor(f"cc_out{c}", [CHUNK // G, C], dt.bfloat16)
              for c in range(NCHUNK)]
    warm_in = nc.dram_tensor("warm_in", [64, 64], dt.bfloat16)
    warm_out = nc.dram_tensor("warm_out", [16, 64], dt.bfloat16)

    with tile.TileContext(nc) as tc:
        with (
            tc.tile_pool(name="persist", bufs=1) as persist,
            tc.tile_pool(name="xsp", bufs=2) as xsp,
            tc.tile_pool(name="statp", bufs=2) as statp,
            tc.tile_pool(name="esp", bufs=2) as esp,
            tc.tile_pool(name="workp", bufs=2) as workp,
            tc.tile_pool(name="wstream", bufs=2) as wstream,
            tc.tile_pool(name="psum", bufs=1, space="PSUM") as psum,
        ):
            # ---------- persistent SBUF ----------
            ident = persist.tile([128, 128], dt.bfloat16)
            ones128 = persist.tile([128, 128], dt.bfloat16)
            ones_row = persist.tile([1, CHUNK], dt.bfloat16)
            eps_col = persist.tile([128, 1], dt.float32)
            cmask_sb = persist.tile([128, 2, 128], dt.bfloat16)
            wqkv_sb = persist.tile([128, NCT, 3 * 64 * HPC], dt.bfloat16)
            wproj_sb = persist.tile([128, 2, C], dt.bfloat16)
            k_sb = persist.tile([128, 2, T], dt.bfloat16)
            v_sb = persist.tile([128, T // 128, (D + 1) * HPC], dt.bfloat16)
            y_sb = persist.tile([128, 2, T], dt.bfloat16)
            x2t_sb = persist.tile([128, G, C], dt.float32)
            x2nT_sb = persist.tile([128, NCT, ROWS], dt.bfloat16)
            h_sb = persist.tile([128, NFT, ROWS], dt.bfloat16)
            if with_bias:
                wqkvb_sb = persist.tile([1, 3 * 64 * HPC], dt.bfloat16)
                bfc_sb = persist.tile([1, FF], dt.bfloat16)
                bmlp_sb = persist.tile([1, C], dt.bfloat16)

            # tiny dummy ReduceScatter fired immediately: absorbs the CC entry
            # barrier + core start-skew before the first real RS
            nc.gpsimd.collective_compute(
                "ReduceScatter", mybir.AluOpType.add,
                replica_groups=[[0, 1, 2, 3], [4, 5, 6, 7]],
                ins=[warm_in.ap().opt()],
                outs=[warm_out.ap().opt()],
            )
            make_identity(nc, ident[:])
            nc.vector.memset(ones128[:], 1.0)
            nc.vector.memset(ones_row[:], 1.0)
            nc.vector.memset(eps_col[:], EPS)
            xs0 = xsp.tile([128, NCT, CHUNK], dt.bfloat16, tag="xs", name="xs0")
            nc.sync.dma_start(cmask_sb[:], cmask_d[:, :, :])
            if with_bias:
                nc.sync.dma_start(wqkvb_sb[:], wqkvb_d[:, :])
                nc.sync.dma_start(bfc_sb[:], bfc_d[:, :])
                nc.sync.dma_start(bmlp_sb[:], bmlp_d[:, :])
            vview = v_sb[:].rearrange("p t (h e) -> p t h e", h=HPC)
            nc.vector.memset(vview[:, :, :, D:D + 1], 1.0)

            WT = ["w0", "w1"]
            wi = [0]

            def wtile(shape, dtype=dt.float32, name=None):
                t = psum.tile(shape, dtype, tag=WT[wi[0] % 2], name=name)
                wi[0] += 1
                return t

            # ---------- MLP prologue: residual + LN2 + transpose ----------
            # split: _dma (issued on the gpsimd queue right after the RS so
            # the sync queue never stalls on an RS-gated DMA), _pre (DVE
            # stats), _chain (scalar rstd + DVE x2n), _tp (PE transposes).
            psh = {}

            def emit_prologue_dma(rt):
                rsl = slice(128 * rt, 128 * (rt + 1))
                xr = workp.tile([128, C], dt.float32, tag="xr", name=f"xr{rt}")
                rsx = workp.tile([128, C], dt.bfloat16, tag="rsx", name=f"rsx{rt}")
                nc.gpsimd.dma_start(xr[:], xrows_d[rsl, :])
                nc.gpsimd.dma_start(rsx[:], cc_out[rt][:, :])
                psh[("dma", rt)] = (xr, rsx)

            def emit_prologue_pre(rt):
                xr, rsx = psh.pop(("dma", rt))
                nc.vector.tensor_add(x2t_sb[:, rt, :], xr[:], rsx[:])
                st = workp.tile([128, 2, 6], dt.float32, tag="st", name=f"st{rt}")
                mv = workp.tile([128, 2], dt.float32, tag="mv", name=f"mv{rt}")
                x2v = x2t_sb[:, rt, :].rearrange("p (s n) -> p s n", s=2)
                for s in range(2):
                    nc.vector.bn_stats(st[:, s, :], x2v[:, s, :])
                nc.vector.bn_aggr(mv[:], st[:])
                psh[rt] = mv

            def emit_prologue_chain(rt):
                mv = psh.pop(rt)
                rstd = workp.tile([128, 1], dt.float32, tag="rstd", name=f"rstd{rt}")
                # 1/std = exp(-0.5*ln(var+eps)) -- stays in the exp table set
                nc.scalar.activation(rstd[:], mv[:, 1:2], AF.Ln,
                                     bias=eps_col[:])
                nc.scalar.activation(rstd[:], rstd[:], AF.Exp, scale=-0.5)
                x2n = workp.tile([128, C], dt.bfloat16, tag="x2n", name=f"x2n{rt}")
                nc.vector.tensor_scalar(x2n[:], x2t_sb[:, rt, :],
                                        mv[:, 0:1], rstd[:],
                                        op0=ALU.subtract, op1=ALU.mult)
                psh[("x2n", rt)] = x2n

            def emit_prologue_tp(rt):
                rsl = slice(128 * rt, 128 * (rt + 1))
                x2n = psh.pop(("x2n", rt))
                for cb in range(NCT):
                    tp = wtile([128, 128], dt.bfloat16, name=f"tp{rt}_{cb}")
                    nc.tensor.transpose(tp[:], x2n[:, 128 * cb:128 * (cb + 1)],
                                        ident[:])
                    nc.vector.tensor_copy(x2nT_sb[:, cb, rsl], tp[:])

            def emit_prologue(rt):
                emit_prologue_pre(rt)
                emit_prologue_chain(rt)
                emit_prologue_tp(rt)

            # ---------- MLP fc tiles over an arbitrary row group ----------
            # rows [r0, r0+nw); moving width nw in {128, 256}
            fc_hold = {}

            def fc_load_w(key, sb):
                wf = wstream.tile([128, NCT, FB], dt.bfloat16, tag="wf", bufs=4,
                                  name=f"wf{key}_{sb}")
                nc.sync.dma_start(wf[:].rearrange("p a b -> p (a b)"),
                                  wfc_d[:, NCT * FB * sb:NCT * FB * (sb + 1)])
                fc_hold[(key, sb)] = wf

            FPS = FB // 128    # ft tiles per slab
            b2_done = [0]      # fc-B ft-completion watermark (for tail gelu)

            def fc_tile(key, r0, nw, ft, fused_gelu):
                if key == "B":
                    b2_done[0] = ft + 1
                wf = fc_hold[(key, ft // FPS)]
                rsl = slice(r0, r0 + nw)
                csl = slice(128 * (ft % FPS), 128 * (ft % FPS + 1))
                ps = wtile([128, nw], name=f"fc{key}_{ft}")
                for ci in range(NCT):
                    nc.tensor.matmul(ps[:], wf[:, ci, csl],
                                     x2nT_sb[:, ci, rsl],
                                     start=(ci == 0),
                                     stop=(ci == NCT - 1 and not with_bias))
                if with_bias:
                    nc.tensor.matmul(ps[:],
                                     bfc_sb[0:1, 128 * ft:128 * (ft + 1)],
                                     ones_row[0:1, 0:nw],
                                     start=False, stop=True)
                if fused_gelu:
                    nc.scalar.activation(h_sb[:, ft, rsl], ps[:], AF.Gelu)
                else:
                    nc.vector.tensor_copy(h_sb[:, ft, rsl], ps[:])

            def emit_fc_group(key, r0, nw, defer=None, fused_gelu=False):
                fc_load_w(key, 0)    # issue the first weight-slab DMA right away
                for ft in range(NFT):
                    def step(ft=ft):
                        if ft % FPS == 0:
                            for nxt in (ft // FPS + 1, ft // FPS + 2):
                                if nxt < NSLAB and (key, nxt) not in fc_hold:
                                    fc_load_w(key, nxt)   # prefetch ahead
                        fc_tile(key, r0, nw, ft, fused_gelu)
                    if defer is None:
                        step()
                    else:
                        defer.append(step)

            # ---------- MLP proj weight stream: one slab per 2 ft ----------
            wm_hold = {}

            def load_wm(key, ftp):
                wm = wstream.tile([128, 2, C], dt.bfloat16, tag="wm", bufs=6,
                                  name=f"wm{key}_{ftp}")
                nc.sync.dma_start(wm[:].rearrange("p a b -> p (a b)"),
                                  wmlp_d[:, 2 * C * ftp:2 * C * (ftp + 1)])
                wm_hold[(key, ftp)] = wm

            def wm_get(key, ft):
                ftp = ft // 2
                if (key, ftp) not in wm_hold:
                    load_wm(key, ftp)
                if ftp + 2 < NFT // 2 and (key, ftp + 2) not in wm_hold:
                    load_wm(key, ftp + 2)     # prefetch 2 slabs ahead
                wm = wm_hold[(key, ftp)]
                if ft % 2 == 1:
                    del wm_hold[(key, ftp)]
                return wm[:, ft % 2, :]

            # ---------- per-row-block MLP proj + residual + store ----------
            def mlp_mm(mps, rt, ft, wmrow):
                for cc in range(2):
                    csl = slice(512 * cc, 512 * (cc + 1))
                    nc.tensor.matmul(mps[cc], h_sb[:, ft, 128 * rt:128 * (rt + 1)],
                                     wmrow[:, csl],
                                     start=(ft == 0),
                                     stop=(ft == NFT - 1 and not with_bias))

            def mlp_fin(mps, rt):
                for cc in range(2):
                    csl = slice(512 * cc, 512 * (cc + 1))
                    if with_bias:
                        nc.tensor.matmul(mps[cc], ones128[0:1, :],
                                         bmlp_sb[0:1, csl],
                                         start=False, stop=True)
                    fin = workp.tile([128, 512], dt.float32, tag="fin",
                                     name=f"fin{rt}_{cc}")
                    nc.vector.tensor_add(fin[:], mps[cc],
                                         x2t_sb[:, rt, csl])
                    nc.gpsimd.dma_start(out_d[slice(128 * rt, 128 * (rt + 1)), csl],
                                        fin[:])

            # mlp accumulators live on the attention-dead sA/sB/y0/y1 PSUM
            # tags so the w0/w1 wtile rotation stays free for fc tiles that
            # interleave with the open mlp accumulation groups
            def alloc_mp2(tag, name):
                t = psum.tile([128, 2, 512], dt.float32, tag=tag, name=name)
                return [t[:, 0, :], t[:, 1, :]]

            # Two fill queues popped into attention units: qkv_q (stats/xn/qkv
            # steps of the NEXT chunk -- must drain before that chunk's
            # attention) and fill_q (fc tiles -- fully opportunistic).
            qkv_q = []
            fill_q = []
            qtiles = {}

            def pop_fill():
                if qkv_q:
                    qkv_q.pop(0)()
                elif fill_q:
                    fill_q.pop(0)()

            def stage_chunk(c, xs):
                """Append LN1-stats + qkv emission steps for chunk c.

                LN1 stats come from DVE bn_stats on row-major x tiles (xfull);
                the per-row mean/rstd columns are transposed once on the PE and
                broadcast to [128, T] via K=1 matmuls.  In the no-bias fast
                path the rstd scale rides on the PSUM->SBUF copies, so the
                in-place xn only subtracts the mean."""
                isl = slice(CHUNK * c, CHUNK * (c + 1))
                sh = {}
                xfs = []
                for tt4 in range(4):
                    xf = statp.tile([128, C], dt.bfloat16, tag="xf", bufs=4,
                                    name=f"xf{c}_{tt4}")
                    nc.sync.dma_start(
                        xf[:], xfull_d[:, 4 * C * c + C * tt4:
                                       4 * C * c + C * (tt4 + 1)])
                    xfs.append(xf)
                qt = statp.tile([128, 2, CHUNK], dt.bfloat16, tag="qt",
                                bufs=2, name=f"qt{c}")
                qtiles[c] = qt
                pack_mu = statp.tile([128, 128], dt.bfloat16, tag="pkm",
                                     bufs=2, name=f"pkm{c}")
                pack_rs = statp.tile([128, 128], dt.bfloat16, tag="pkr",
                                     bufs=2, name=f"pkr{c}")
                rcol = statp.tile([128, 4], dt.float32, tag="rcol", bufs=2,
                                  name=f"rcol{c}")

                def bn_step(tt4):
                    xf = xfs[tt4]
                    st = statp.tile([128, 2, 6], dt.float32, tag="bst", bufs=2,
                                    name=f"bst{c}_{tt4}")
                    mv = statp.tile([128, 2], dt.float32, tag="bmv", bufs=2,
                                    name=f"bmv{c}_{tt4}")
                    xfv = xf[:].rearrange("p (s n) -> p s n", s=2)
                    for sb2 in range(2):
                        nc.vector.bn_stats(st[:, sb2, :], xfv[:, sb2, :])
                    nc.vector.bn_aggr(mv[:], st[:])
                    # rstd = exp(-0.5*ln(var+eps)) -- stays in the exp table
                    nc.scalar.activation(rcol[:, tt4:tt4 + 1], mv[:, 1:2],
                                         AF.Ln, bias=eps_col[:])
                    nc.scalar.activation(rcol[:, tt4:tt4 + 1],
                                         rcol[:, tt4:tt4 + 1], AF.Exp,
                                         scale=-0.5)
                    nc.vector.tensor_copy(
                        pack_mu[:, 32 * tt4:32 * tt4 + 1], mv[:, 0:1])
                    nc.vector.tensor_copy(
                        pack_rs[:, 32 * tt4:32 * tt4 + 1],
                        rcol[:, tt4:tt4 + 1])

                def tb_step():
                    tpm = wtile([128, 128], dt.bfloat16, name=f"stm{c}")
                    nc.tensor.transpose(tpm[:], pack_mu[:], ident[:])
                    tpr = wtile([128, 128], dt.bfloat16, name=f"str{c}")
                    nc.tensor.transpose(tpr[:], pack_rs[:], ident[:])
                    srow = statp.tile([1, 8, 128], dt.bfloat16, tag="srow",
                                      bufs=2, name=f"srow{c}")
                    for tt4 in range(4):
                        nc.vector.tensor_copy(srow[0:1, 2 * tt4, :],
                                              tpm[32 * tt4:32 * tt4 + 1, :])
                        nc.vector.tensor_copy(srow[0:1, 2 * tt4 + 1, :],
                                              tpr[32 * tt4:32 * tt4 + 1, :])
                    mu_ps = wtile([128, CHUNK], name=f"mups{c}")
                    rs_ps = wtile([128, CHUNK], name=f"rsps{c}")
                    for tt4 in range(4):
                        csl = slice(128 * tt4, 128 * (tt4 + 1))
                        nc.tensor.matmul(mu_ps[:, csl], ones128[0:1, :],
                                         srow[0:1, 2 * tt4, :],
                                         start=True, stop=True,
                                         skip_group_check=True)
                        nc.tensor.matmul(rs_ps[:, csl], ones128[0:1, :],
                                         srow[0:1, 2 * tt4 + 1, :],
                                         start=True, stop=True,
                                         skip_group_check=True)
                    mu_t = statp.tile([128, CHUNK], dt.bfloat16, tag="mu",
                                      name=f"mu{c}")
                    rsig = statp.tile([128, CHUNK], dt.bfloat16, tag="rsig",
                                      name=f"rsig{c}")
                    nc.vector.tensor_copy(mu_t[:], mu_ps[:])
                    nc.vector.tensor_copy(rsig[:], rs_ps[:])
                    sh["mu"], sh["rsig"] = mu_t, rsig

                def xn_step(g):             # 2 steps x 4 ci, in-place
                    for ci in range(4 * g, 4 * g + 4):
                        nc.vector.tensor_sub(xs[:, ci, :], xs[:, ci, :],
                                             sh["mu"][:])
                        if with_bias:
                            nc.vector.tensor_mul(xs[:, ci, :], xs[:, ci, :],
                                                 sh["rsig"][:])

                def qk_step(blk, s):
                    dsl = (slice(None), s, slice(0, CHUNK)) if blk == 0 \
                        else (slice(None), s, isl)
                    dst = qt if blk == 0 else k_sb
                    cols = slice(256 * blk + 128 * s, 256 * blk + 128 * (s + 1))
                    ps = wtile([128, CHUNK], name=f"qk{c}_{blk}_{s}")
                    for ci in range(NCT):
                        nc.tensor.matmul(ps[:], wqkv_sb[:, ci, cols],
                                         xs[:, ci, :],
                                         start=(ci == 0),
                                         stop=(ci == NCT - 1 and not with_bias))
                    if with_bias:
                        nc.tensor.matmul(ps[:], wqkvb_sb[0:1, cols],
                                         ones_row[0:1, :],
                                         start=False, stop=True)
                        nc.vector.tensor_copy(dst[dsl], ps[:])
                    else:
                        nc.vector.tensor_mul(dst[dsl], ps[:],
                                             sh["rsig"][:])

                def v_step(tt4):
                    tt = 4 * c + tt4
                    tsl = slice(128 * tt4, 128 * (tt4 + 1))
                    ps = wtile([128, 64 * HPC], name=f"v{c}_{tt4}")
                    for ci in range(NCT):
                        nc.tensor.matmul(ps[:], xs[:, ci, tsl],
                                         wqkv_sb[:, ci, 512:768],
                                         start=(ci == 0),
                                         stop=(ci == NCT - 1 and not with_bias))
                    if with_bias:
                        nc.tensor.matmul(ps[:], ones_row[0:1, 0:128],
                                         wqkvb_sb[0:1, 512:768],
                                         start=False, stop=True)
                        nc.vector.tensor_copy(
                            vview[:, tt, :, 0:D],
                            ps[:].rearrange("p (h e) -> p h e", e=D))
                    else:
                        nc.vector.tensor_scalar_mul(
                            vview[:, tt, :, 0:D],
                            ps[:].rearrange("p (h e) -> p h e", e=D),
                            rcol[:, tt4:tt4 + 1])

                for tt4 in range(4):
                    qkv_q.append(lambda tt4=tt4: bn_step(tt4))
                qkv_q.append(tb_step)
                for g in range(2):
                    qkv_q.append(lambda g=g: xn_step(g))
                for blk in (0, 1):
                    for s in range(2):
                        qkv_q.append(lambda blk=blk, s=s: qk_step(blk, s))
                for tt4 in range(CHUNK // 128):
                    qkv_q.append(lambda tt4=tt4: v_step(tt4))

            # a few junk matmuls during the initial x DMA: ~3.5us of PE busy
            # flips the HAM clock gate to its fast state before real work
            warm_ps = psum.tile([128, CHUNK], dt.float32, tag="w0",
                                name="warm_ps")
            for _ in range(40):
                nc.tensor.matmul(warm_ps[:], ones128[0:1, :], ones_row[0:1, :],
                                 start=True, stop=True)

            # =======================  main chunk loop  =======================
            for c in range(NCHUNK):
                isl = slice(CHUNK * c, CHUNK * (c + 1))

                # chunk c's stats/qkv must be fully emitted before its attention
                if c == 0:
                    # interleave chunk-0 and chunk-1 qkv pipelines so chunk-1
                    # stats matmuls fill chunk-0's stats->chain->xn latency.
                    # Order is deadlock-safe wrt the w0/w1 PSUM tag rotation:
                    # B's chain (B4) must be emitted before A's qk/v wtiles.
                    stage_chunk(0, xs0)
                    # x/weight streams ordered behind chunk-0's xfull tiles
                    nc.sync.dma_start(xs0[:].rearrange("p a b -> p (a b)"),
                                      xT_d[:, 0:NCT * CHUNK])
                    nc.sync.dma_start(wqkv_sb[:].rearrange("p a b -> p (a b)"),
                                      wqkv_d[:, :])
                    qA = list(qkv_q); qkv_q.clear()
                    xs1 = xsp.tile([128, NCT, CHUNK], dt.bfloat16, tag="xs",
                                   name="xs1")
                    nc.sync.dma_start(xs1[:].rearrange("p a b -> p (a b)"),
                                      xT_d[:, NCT * CHUNK:2 * NCT * CHUNK])
                    nc.sync.dma_start(wproj_sb[:, 0, :], wproj_d[0:128, :])
                    nc.sync.dma_start(wproj_sb[:, 1, :], wproj_d[128:256, :])
                    stage_chunk(1, xs1)
                    qB = list(qkv_q); qkv_q.clear()
                    for step in (qA[0:5] + qB[0:2] + qA[5:7] + qB[2:5]
                                 + qA[7:15]):
                        step()
                    qkv_q.extend(qB[5:])
                while qkv_q:
                    qkv_q.pop(0)()

                # stage the NEXT chunk: stream its x^T, queue its steps
                if c + 1 < NCHUNK and c >= 1:
                    xs_n = xsp.tile([128, NCT, CHUNK], dt.bfloat16, tag="xs",
                                    name=f"xs{c + 1}")
                    nc.sync.dma_start(
                        xs_n[:].rearrange("p a b -> p (a b)"),
                        xT_d[:, NCT * CHUNK * (c + 1):NCT * CHUNK * (c + 2)])
                    stage_chunk(c + 1, xs_n)

                # ---- attention for this chunk ----
                njt = 4 * c + 4
                for hp in range(2):
                    ys = [psum.tile([D + 1, CHUNK], dt.float32, tag=f"y{u}",
                                    name=f"ys{c}_{hp}_{u}") for u in range(2)]
                    for jt in range(njt):
                        # at attn(2) hp1-jt4, RS(0) and RS(1) have finished:
                        # prologues 0+1, then fc-A tiles fill the units
                        if c == 2 and hp == 1 and jt == 4:
                            emit_prologue(0)
                            emit_prologue(1)
                            emit_fc_group("A", 0, 256, defer=fill_q)
                        # late hp0: RS(2) is done -> prologue(2) + fc-B2
                        # tiles for rows 256-383 keep the fill queue stocked
                        if c == 3 and hp == 0 and jt == 12:
                            emit_prologue(2)
                            emit_fc_group("B", 256, 128, defer=fill_q)
                        # prebuffer mlp-proj weight slabs for the tail, two
                        # at a time so the slab feed for fills never starves
                        if c == 3 and hp == 1 and jt in (0, 6, 12):
                            load_wm("m", jt // 3)
                            load_wm("m", jt // 3 + 1)
                        jsl = slice(128 * jt, 128 * (jt + 1))
                        off = 128 * (jt - 4 * c) if jt >= 4 * c else 0
                        sp = psum.tile([128, 2, CHUNK], dt.float32,
                                       tag=("sA" if jt % 2 == 0 else "sB"),
                                       name=f"sp{c}_{hp}_{jt}")
                        for u in range(2):
                            r = slice(64 * u, 64 * (u + 1))
                            nc.tensor.matmul(
                                sp[:, u, off:CHUNK],
                                k_sb[r, hp, jsl],
                                qtiles[c][r, hp, off:CHUNK],
                                start=True, stop=True,
                                tile_position=(64 * u, 0))
                        es = esp.tile([128, 2, CHUNK], dt.bfloat16, tag="es",
                                      name=f"es{c}_{hp}_{jt}")
                        nc.scalar.activation(es[:, :, off:CHUNK],
                                             sp[:, :, off:CHUNK], AF.Exp)
                        if jt >= 4 * c:
                            nc.vector.tensor_mul(es[:, :, off:off + 128],
                                                 es[:, :, off:off + 128],
                                                 cmask_sb[:])
                        for u in range(2):
                            h_ = 2 * hp + u
                            nc.tensor.matmul(
                                ys[u][:, off:CHUNK],
                                v_sb[:, jt, (D + 1) * h_:(D + 1) * (h_ + 1)],
                                es[:, u, off:CHUNK],
                                start=(jt == 0), stop=(jt == njt - 1),
                                skip_group_check=True)
                        pop_fill()   # interleave a staged qkv/fc step
                    for u in range(2):
                        ysb = workp.tile([D + 1, CHUNK], dt.bfloat16, tag="ysb",
                                         name=f"ysb{c}_{hp}_{u}")
                        nc.vector.tensor_copy(ysb[:], ys[u][:])
                        # 1/denom = exp(-ln(d)) on the [1,512] row (exp-set
                        # resident), then matmul-broadcast across partitions
                        rln = workp.tile([D + 1, CHUNK], dt.float32, tag="rln", bufs=1,
                                         name=f"rln{c}_{hp}_{u}")
                        nc.scalar.activation(rln[D:D + 1, :], ysb[D:D + 1, :],
                                             AF.Ln)
                        rinv = workp.tile([D + 1, CHUNK], dt.bfloat16, tag="rinv", bufs=1,
                                          name=f"rinv{c}_{hp}_{u}")
                        nc.scalar.activation(rinv[D:D + 1, :], rln[D:D + 1, :],
                                             AF.Exp, scale=-1.0)
                        dbc = wtile([128, CHUNK], name=f"dbc{c}_{hp}_{u}")
                        nc.tensor.matmul(dbc[:], ones128[D:D + 1, :],
                                         rinv[D:D + 1, :], start=True, stop=True)
                        nc.vector.tensor_mul(y_sb[64 * u:64 * (u + 1), hp, isl],
                                             ysb[0:D, :], dbc[0:D, :])

                # ---- attention proj for this chunk ----
                for tt4 in range(CHUNK // 128):
                    t0 = CHUNK * c + 128 * tt4
                    for cc in range(2):
                        csl = slice(512 * cc, 512 * (cc + 1))
                        pp = wtile([128, 512], name=f"pp{c}_{tt4}_{cc}")
                        for hp in range(2):
                            nc.tensor.matmul(pp[:], y_sb[:, hp, t0:t0 + 128],
                                             wproj_sb[:, hp, csl],
                                             start=(hp == 0), stop=(hp == 1))
                        ob = workp.tile([128, 512], dt.bfloat16, tag="ob",
                                        name=f"ob{c}_{tt4}_{cc}")
                        nc.vector.tensor_copy(ob[:], pp[:])
                        nc.gpsimd.dma_start(
                            cc_in[c][128 * tt4:128 * (tt4 + 1), csl], ob[:])

                # ---- ReduceScatter for this chunk's rows ----
                nc.gpsimd.collective_compute(
                    "ReduceScatter", mybir.AluOpType.add,
                    replica_groups=[[0, 1, 2, 3], [4, 5, 6, 7]],
                    ins=[cc_in[c].ap().opt()],
                    outs=[cc_out[c].ap().opt()],
                )
                # pre-issue this chunk's prologue DMAs right behind the RS:
                # they carry the RS sem-wait, and nothing urgent sits behind
                # them on the gpsimd queue
                emit_prologue_dma(c)

            # ============================  tail  ============================
            # mlp row-blocks 0-2 interleaved with the remaining fc-B2 fills
            # (spreads their slab-DMA demand) and with bulk gelu of rows
            # 0-383 in 8-ft blocks; all of this runs during RS(3).
            mps012 = [alloc_mp2("sA", "mp0"), alloc_mp2("sB", "mp1"),
                      [psum.tile([128, 512], dt.float32, tag="y0", name="mp2_0"),
                       psum.tile([128, 512], dt.float32, tag="y1", name="mp2_1")]]

            def drain_b2(ft_needed):
                while fill_q and b2_done[0] < ft_needed:
                    fill_q.pop(0)()

            drain_b2(8)
            nc.scalar.activation(h_sb[:, 0:8, 0:384], h_sb[:, 0:8, 0:384],
                                 AF.Gelu)
            for ft in range(NFT):
                if fill_q:
                    fill_q.pop(0)()
                if ft % 8 == 7 and ft < NFT - 1:
                    fb = ft // 8 + 1
                    drain_b2(8 * (fb + 1))
                    nc.scalar.activation(h_sb[:, 8 * fb:8 * (fb + 1), 0:384],
                                         h_sb[:, 8 * fb:8 * (fb + 1), 0:384],
                                         AF.Gelu)
                if ft == 26:
                    fc_load_w("C", 0)
                    fc_load_w("C", 1)
                if ft == 29:
                    load_wm("m3", 0)
                    load_wm("m3", 1)
                wmrow = wm_get("m", ft)
                for rt in range(3):
                    mlp_mm(mps012[rt], rt, ft, wmrow)
            while fill_q:
                fill_q.pop(0)()
            for rt in range(3):
                mlp_fin(mps012[rt], rt)
            # prologue(3) DVE stats: emitted only now so the RS(3)-gated data
            # dependency never stalls the B2 h-copies in the DVE FIFO
            emit_prologue_pre(3)
            # rows 384-511: LN2 chain (exp-set reload), transposes, then fc-C
            # with fused gelu interleaved per-ft with mlp row-block 3.
            emit_prologue_chain(3)
            emit_prologue_tp(3)
            mps3 = alloc_mp2("sA", "mp3")
            for ft in range(NFT):
                if ft % FPS == 0:
                    for nxt in (ft // FPS + 1, ft // FPS + 2):
                        if nxt < NSLAB and ("C", nxt) not in fc_hold:
                            fc_load_w("C", nxt)
                fc_tile("C", 384, 128, ft, fused_gelu=True)
                wmrow = wm_get("m3", ft)
                mlp_mm(mps3, 3, ft, wmrow)
            mlp_fin(mps3, 3)

    _split_excess_waits(nc, mybir)
    return nc


def _get_nc(with_bias):
    key = ("nc", with_bias)
    if key not in _cache:
        _cache[key] = _build(with_bias)
    return _cache[key]


def make_in_maps(inputs, with_bias):
    import ml_dtypes
    bf16 = ml_dtypes.bfloat16
    x = np.asarray(inputs["x"], np.float32)
    w_qkv = np.asarray(inputs["w_qkv"], np.float32)
    w_attn_proj = np.asarray(inputs["w_attn_proj"], np.float32)
    ln1_w = np.asarray(inputs["ln1_w"], np.float32)
    ln1_b = np.asarray(inputs["ln1_b"], np.float32)
    ln2_w = np.asarray(inputs["ln2_w"], np.float32)
    ln2_b = np.asarray(inputs["ln2_b"], np.float32)
    w_fc = np.asarray(inputs["w_fc"], np.float32)
    b_fc = np.asarray(inputs["b_fc"], np.float32)
    w_mlp_proj = np.asarray(inputs["w_mlp_proj"], np.float32)
    b_mlp_proj = np.asarray(inputs["b_mlp_proj"], np.float32)

    wfc_in = (ln2_w[:, None] * w_fc).astype(bf16)           # [C, FF]
    # wfc re-layout: [128, (slab, ci, fb)]  slab = f // FB
    wfc_r = wfc_in.reshape(NCT, 128, NSLAB, FB)             # [ci, p, slab, fb]
    wfc_r = np.ascontiguousarray(wfc_r.transpose(1, 2, 0, 3)).reshape(128, -1)
    # wmlp re-layout: [128, (ft, c)]
    wmlp_r = w_mlp_proj.astype(bf16).reshape(NFT, 128, C)
    wmlp_r = np.ascontiguousarray(wmlp_r.transpose(1, 0, 2)).reshape(128, -1)

    jj = np.arange(128)[:, None]
    ii = np.arange(128)[None, :]
    cm1 = (ii >= jj).astype(np.float32)
    cmask = np.stack([cm1, cm1], axis=1).astype(bf16)   # [128, 2, 128]

    in_maps = []
    for core in range(N_CORES):
        b, g = divmod(core, G)
        hsl = slice(256 * g, 256 * (g + 1))
        raw768 = np.concatenate([w_qkv[:, :C][:, hsl] * QSCALE,
                                 w_qkv[:, C:2 * C][:, hsl],
                                 w_qkv[:, 2 * C:][:, hsl]], axis=1)
        rowidx = np.concatenate([np.arange(CHUNK * c + 128 * g,
                                           CHUNK * c + 128 * (g + 1))
                                 for c in range(NCHUNK)])
        # xT re-layout: [128, (chunk, ci, t)]
        xT = x[b].T.astype(bf16)                            # [C, T]
        xT_r = xT.reshape(NCT, 128, NCHUNK, CHUNK)          # [ci, p, c, t]
        xT_r = np.ascontiguousarray(xT_r.transpose(1, 2, 0, 3)).reshape(128, -1)
        # xfull re-layout: [128, (chunk, row-tile, C)] row-major x
        xfull_r = x[b].astype(bf16).reshape(NCHUNK * 4, 128, C)
        xfull_r = np.ascontiguousarray(
            xfull_r.transpose(1, 0, 2)).reshape(128, -1)
        # wqkv re-layout: [128, (ci, col)]
        wq = (ln1_w[:, None] * raw768).astype(bf16)         # [C, 768]
        wq_r = np.ascontiguousarray(
            wq.reshape(NCT, 128, 3 * 64 * HPC).transpose(1, 0, 2)).reshape(128, -1)
        m = {
            "cmask": cmask,
            "xT": xT_r,
            "xfull": xfull_r,
            "wqkv": wq_r,
            "wproj": w_attn_proj[hsl, :].astype(bf16),
            "xrows": np.ascontiguousarray(x[b, rowidx, :]),
            "wfc": wfc_r,
            "wmlp": wmlp_r,
        }
        if with_bias:
            m["wqkvb"] = (ln1_b @ raw768)[None, :].astype(bf16)
            m["bfc"] = (b_fc + ln2_b @ w_fc)[None, :].astype(bf16)
            m["bmlp"] = b_mlp_proj[None, :].astype(bf16)
        in_maps.append(m)
    return in_maps


def assemble_out(results):
    out = np.empty((B, T, C), np.float32)
    for core in range(N_CORES):
        b, g = divmod(core, G)
        for c in range(NCHUNK):
            out[b, CHUNK * c + 128 * g:CHUNK * c + 128 * (g + 1), :] = \
                results[core]["out"][128 * c:128 * (c + 1), :]
    return out


def _setup_trace_shims():
    """Register the NTFF profile hook (missing antenv.axon_hooks on this image)
    so run_bass_kernel_spmd(trace=True) can report exec_time_ns."""
    import sys, types
    if "antenv.axon_hooks" not in sys.modules:
        mod = types.ModuleType("antenv.axon_hooks")
        hook = {}
        mod.set_axon_ntff_profile_hook = lambda h: hook.__setitem__("h", h)
        mod.get_axon_ntff_profile_hook = lambda: hook.get("h")
        sys.modules["antenv.axon_hooks"] = mod
        try:
            from trn_agent_boot.trn_boot import _ntff_profile_via_ctypes
            mod.set_axon_ntff_profile_hook(
                _ntff_profile_via_ctypes("/opt/axon/libaxon_pjrt.so"))
        except Exception:
            pass
    import concourse.bass_utils as bu
    bu.upload_artifacts = lambda tmpdir: tmpdir


def kernel(**inputs):
    trace = bool(os.environ.get("KERNEL_TRACE"))
    if trace:
        _setup_trace_shims()
    from concourse.bass_utils import run_bass_kernel_spmd

    with_bias = not all(
        float(np.abs(np.asarray(inputs[k], np.float32)).max()) == 0.0
        for k in ("ln1_b", "ln2_b", "b_fc", "b_mlp_proj"))
    in_maps = make_in_maps(inputs, with_bias)
    nc = _get_nc(with_bias)
    res = run_bass_kernel_spmd(nc, in_maps, core_ids=list(range(N_CORES)),
                               trace=trace)
    if trace:
        _cache["exec_time_ns"] = res.exec_time_ns
    return assemble_out(res.results)


if __name__ == "__main__":
    nc = _get_nc(False)
    print("built OK; instructions:", len(nc.inst_map))
